# revision 1
# baseline (speedup 1.0000x reference)
"""CopyTokenDecoder Trainium2 kernel (v3).

Sharding: data-parallel over batch B=8 -> one NeuronCore per batch element.

Vocab math (per core): out[t,v] = log(gen*softmax(logits) + copy + eps).
Non-copy columns: out = logits + rowconst, rowconst = log(gen/sv).
The final LayerNorm is folded into the vocab matmul: the embedding is
pre-multiplied by ln2_g on the host, the 1/std factor rides the ACT scale /
DVE multiply, so the device computes logits' = centered(h) @ (W*g2)^T and
out = rstd2*logits' + rowconst.  W_emb stays resident in SBUF as fp8
(DoubleRow matmuls).  Pass A: matmul -> exp(rstd2*psum) accum -> sv.
Pass B: matmul again, fused scale+add on DVE/ACT/Pool round-robin, bf16 out.
Copy columns (<=512 unique ids) go through a compact [T,512] side path and
are scattered into the output on the host.  rr (attention softmax denom) is
the row-sum of the compact delta matmul.
ACT functions are sequenced to need only 6 table loads (Exp/Square clusters,
one Ln cluster at the end).
"""

from contextlib import ExitStack

import numpy as np
import ml_dtypes

import concourse.tile as tile
from concourse import bacc, mybir
from concourse.bass_utils import run_bass_kernel_spmd
from concourse.masks import make_identity

F32 = mybir.dt.float32
BF16 = mybir.dt.bfloat16
FP8 = mybir.dt.float8e4
AF = mybir.ActivationFunctionType
OP = mybir.AluOpType
PM = mybir.MatmulPerfMode
BF = ml_dtypes.bfloat16
F8 = ml_dtypes.float8_e4m3

T, B, S, D, F, V = 256, 8, 512, 512, 2048, 32000
P = 128
DSCALE = float(D) ** -0.5
NEG = -1.0e30
TT = 2                      # t-tiles of 128
NCHUNK = 16                 # vocab chunks of 2048 (last covers 1280)
CHUNK = 2048
NCOL = 512                  # compact copy-column capacity
EPS_LN = 1e-5
EPS_LOG = 1e-12

_CACHE = {}


def _cw(c):
    return CHUNK if c < NCHUNK - 1 else V - (NCHUNK - 1) * CHUNK


def _subwidths(c):
    w = _cw(c)
    out = []
    while w > 0:
        out.append(min(512, w))
        w -= 512
    return out


def _build(has_b2g):
    nc = bacc.Bacc("TRN2", target_bir_lowering=False, debug=False,
                   enable_asserts=False, num_devices=B)

    def din(name, shape, dt):
        return nc.dram_tensor(name, shape, dt, kind="ExternalInput").ap()

    # per-core tensors
    outsT_d = din("outsT", [D, T], BF16)
    outs_tok_d = din("outs_tok", [T, D], BF16)
    memT_d = din("memT", [D, S], BF16)
    maskrow_d = din("maskrow", [1, S], BF16)
    mcol_d = din("mcol", [S, NCOL], FP8)
    wcolsT_d = din("wcolsT", [D, NCOL], FP8)
    # shared weights
    wqT_d = din("wqT", [D, D], BF16)
    wkT_d = din("wkT", [D, D], BF16)
    wvT_d = din("wvT", [D, D], BF16)
    woT_d = din("woT", [D, D], BF16)
    w1T_d = din("w1T", [D, F], FP8)
    w2T_d = din("w2T", [F, D], FP8)
    wemb8_d = din("wemb8", [NCHUNK, P, 4 * CHUNK], FP8)
    bq_d = din("bq_c", [P, 4], F32)
    bk_d = din("bk_c", [P, 4], F32)
    bvrow_d = din("bv_row", [1, D], BF16)
    bo_tok_d = din("bo_tok", [P, D], BF16)
    b1_d = din("b1_c", [P, 16], F32)
    b2_d = din("b2_c", [P, 4], F32)
    wd1_d = din("wd1_tok", [P, D], BF16)
    wdn2_d = din("wdn2_tok", [P, D], BF16)
    nbdd_d = din("nbddiff", [P, 1], F32)
    ones_d = din("ones_row", [1, T], BF16)
    if has_b2g:
        bwrow_d = din("bw_row", [1, NCHUNK * CHUNK], F32)
        bwcol_d = din("bwcol_row", [1, NCOL], F32)

    out_d = nc.dram_tensor("out", [T, V], BF16, kind="ExternalOutput").ap()
    out_r = out_d.rearrange("(th tl) v -> tl th v", tl=P)
    fix_d = nc.dram_tensor("fix", [T, NCOL], BF16, kind="ExternalOutput").ap()
    fix_r = fix_d.rearrange("(th tl) j -> tl th j", tl=P)

    r3 = lambda ap, inner: ap.rearrange("(hi lo) x -> lo hi x", lo=P)

    with tile.TileContext(nc) as tc, ExitStack() as octx:
        cpool = octx.enter_context(tc.tile_pool(name="cpool", bufs=1))
        # ---- persistent tiles ----
        ident_f = cpool.tile([P, P], F32, tag="ident_f")
        make_identity(nc, ident_f[:])
        ident_b = cpool.tile([P, P], BF16, tag="ident_b")
        nc.vector.tensor_copy(ident_b[:], ident_f[:])
        rr = cpool.tile([P, TT], F32, tag="rr")          # 1/sum_s exp(scores)
        cgrr = cpool.tile([P, TT], F32, tag="cgrr")      # cg * rr
        gg = cpool.tile([P, TT], F32, tag="gg")          # gen gate
        rstd2 = cpool.tile([P, TT], F32, tag="rstd2")    # 1/std of LN2
        sv_parts = cpool.tile([P, TT * NCHUNK], F32, tag="sv_parts")
        svs = cpool.tile([P, TT], F32, tag="svs")
        ggsv = cpool.tile([P, TT], F32, tag="ggsv")      # gg / sv
        rowconst = cpool.tile([P, TT], F32, tag="rowconst")
        x2T8 = cpool.tile([P, 4, T], FP8, tag="x2T8")
        delta_tok = cpool.tile([P, TT, NCOL], BF16, tag="delta_tok")
        onesr = cpool.tile([1, T], BF16, tag="onesr")
        nc.sync.dma_start(onesr[:], ones_d[:])
        maskrow = cpool.tile([1, S], BF16, tag="maskrow")
        nc.sync.dma_start(maskrow[:], maskrow_d[:])
        eps_ln_c = cpool.tile([P, 1], F32, tag="eps_ln_c")
        nc.gpsimd.memset(eps_ln_c[:], EPS_LN)
        eps_log_c = cpool.tile([P, 1], F32, tag="eps_log_c")
        nc.gpsimd.memset(eps_log_c[:], EPS_LOG)
        if has_b2g:
            bw_row = cpool.tile([1, NCHUNK * CHUNK], F32, tag="bw_row")
            nc.sync.dma_start(bw_row[:], bwrow_d[:])
            bwc_row = cpool.tile([1, NCOL], F32, tag="bwc_row")
            nc.sync.dma_start(bwc_row[:], bwcol_d[:])

        # ================= front-end =================
        fctx = ExitStack()
        fw = fctx.enter_context(tc.tile_pool(name="fw", bufs=1))
        fe = fctx.enter_context(tc.tile_pool(name="fe", bufs=1))

        def load(pool, dram_ap, shape, tag):
            t_ = pool.tile(shape, dram_ap.dtype, tag=tag, name=tag)
            nc.sync.dma_start(t_[:], dram_ap)
            return t_

        # critical-path loads first, then the rest, then resident wemb
        outsT = load(fw, r3(outsT_d, T), [P, 4, T], "outsT")
        memT = load(fw, r3(memT_d, S), [P, 4, S], "memT")
        wqT = load(fw, r3(wqT_d, D), [P, 4, D], "wqT")
        wkT = load(fw, r3(wkT_d, D), [P, 4, D], "wkT")
        wvT = load(fw, r3(wvT_d, D), [P, 4, D], "wvT")
        woT = load(fw, r3(woT_d, D), [P, 4, D], "woT")
        bq_c = load(fw, bq_d, [P, 4], "bq_c")
        bk_c = load(fw, bk_d, [P, 4], "bk_c")
        bv_row = load(fw, bvrow_d, [1, D], "bv_row")
        mcol = load(fw, r3(mcol_d, NCOL), [P, 4, NCOL], "mcol")
        w1T = load(fw, r3(w1T_d, F), [P, 4, F], "w1T")
        w2T = load(fw, r3(w2T_d, D), [P, 16, D], "w2T")
        b1_c = load(fw, b1_d, [P, 16], "b1_c")
        b2_c = load(fw, b2_d, [P, 4], "b2_c")
        outs_tok = load(fe, outs_tok_d.rearrange("(th tl) d -> tl th d", tl=P),
                        [P, TT, D], "outs_tok")
        bo_tok = load(fe, bo_tok_d, [P, D], "bo_tok")
        wd1_tok = load(fe, wd1_d, [P, D], "wd1_tok")
        wdn2_tok = load(fe, wdn2_d, [P, D], "wdn2_tok")
        nbddiff = load(fe, nbdd_d, [P, 1], "nbddiff")
        wcolsT = cpool.tile([P, 4, NCOL], FP8, tag="wcolsT")
        nc.sync.dma_start(wcolsT[:], r3(wcolsT_d, NCOL))

        # resident fp8 embedding: 16 x [P, 4, CHUNK] (128KB/partition)
        wembs = []
        for c in range(NCHUNK):
            w = cpool.tile([P, 4, CHUNK], FP8, tag=f"wemb{c}", name=f"wemb{c}")
            nc.sync.dma_start(w[:], wemb8_d[c].rearrange("p (hi v) -> p hi v",
                                                         v=CHUNK))
            wembs.append(w)

        attn_tok = fe.tile([P, TT, D], BF16, tag="attn_tok")

        # --- layernorm helpers: no centered-copy materialization.
        # stats: negmu = -mean(src); ss = sum((src-mu)^2) via Square's bias.
        # rstd = (ss/D+eps)^-0.5 via Ln+Exp (batched, one table cluster).
        def ln_stats(pool, scr_pool, src_ap, negmu_col, ss_col, nm):
            nc.vector.reduce_sum(negmu_col, src_ap, axis=mybir.AxisListType.X)
            nc.vector.tensor_scalar(out=negmu_col, in0=negmu_col,
                                    scalar1=-1.0 / D, scalar2=None, op0=OP.mult)
            scr = scr_pool.tile([P, D], F32, tag="fscr", name="ln_scr")
            nc.scalar.activation(scr[:], src_ap, AF.Square, bias=negmu_col,
                                 accum_out=ss_col)

        def ln_rstd(rstd_dst, ss_ap):
            nc.scalar.activation(rstd_dst, ss_ap, AF.Ln, bias=eps_ln_c[:, :1],
                                 scale=1.0 / D)
            nc.scalar.activation(rstd_dst, rstd_dst, AF.Exp, scale=-0.5)

        # ---------------- stage A: attention ----------------
        with ExitStack() as actx:
            fa = actx.enter_context(tc.tile_pool(name="fa", bufs=1))
            fad = actx.enter_context(tc.tile_pool(name="fad", bufs=2))
            fp = actx.enter_context(tc.tile_pool(name="fp", bufs=4, space="PSUM"))
            fp5 = actx.enter_context(tc.tile_pool(name="fp5", bufs=2, space="PSUM"))
            fpb = actx.enter_context(tc.tile_pool(name="fab", bufs=2,
                                                  space="PSUM"))

            # PE warm-up while input DMAs land
            wu = fa.tile([P, 512], BF16, tag="wu")
            nc.gpsimd.memset(wu[:], 0.0)
            wu_ps = fp5.tile([P, 512], F32, tag="ps512", space="PSUM")
            for i in range(20):
                nc.tensor.matmul(wu_ps[:], lhsT=ident_b[:], rhs=wu[:],
                                 start=(i == 0), stop=(i == 19))

            qT = fa.tile([P, 4, T], BF16, tag="qT")
            for ho in range(4):
                ps = fp.tile([P, T], F32, tag="ps256", space="PSUM")
                for k in range(4):
                    nc.tensor.matmul(ps[:], lhsT=wqT[:, k, ho * P:(ho + 1) * P],
                                     rhs=outsT[:, k, :], start=(k == 0),
                                     stop=(k == 3))
                nc.vector.tensor_scalar(out=qT[:, ho, :], in0=ps[:],
                                        scalar1=bq_c[:, ho:ho + 1],
                                        scalar2=DSCALE, op0=OP.add, op1=OP.mult)
            kT = fa.tile([P, 4, S], BF16, tag="kT")
            for ho in range(4):
                ps = fp5.tile([P, 512], F32, tag="ps512", space="PSUM")
                for k in range(4):
                    nc.tensor.matmul(ps[:], lhsT=wkT[:, k, ho * P:(ho + 1) * P],
                                     rhs=memT[:, k, :], start=(k == 0),
                                     stop=(k == 3))
                nc.vector.tensor_scalar(out=kT[:, ho, :], in0=ps[:],
                                        scalar1=bk_c[:, ho:ho + 1],
                                        scalar2=None, op0=OP.add)
            v_sb = fa.tile([P, 4, D], BF16, tag="v_sb")
            for sc in range(4):
                ps = fp5.tile([P, 512], F32, tag="ps512", space="PSUM")
                for k in range(4):
                    nc.tensor.matmul(ps[:], lhsT=memT[:, k, sc * P:(sc + 1) * P],
                                     rhs=wvT[:, k, :], start=(k == 0), stop=False)
                nc.tensor.matmul(ps[:], lhsT=onesr[:1, :P], rhs=bv_row[:],
                                 start=False, stop=True)
                if sc % 2 == 0:
                    nc.vector.tensor_copy(v_sb[:, sc, :], ps[:])
                else:
                    nc.scalar.copy(v_sb[:, sc, :], ps[:])

            # scoresT -> exp_c (s-major)
            exp_c = fa.tile([P, 4, T], BF16, tag="exp_c")
            for sc in range(4):
                ps = fp.tile([P, T], F32, tag="ps256", space="PSUM")
                for k in range(4):
                    nc.tensor.matmul(ps[:], lhsT=kT[:, k, sc * P:(sc + 1) * P],
                                     rhs=qT[:, k, :], start=(k == 0), stop=False)
                nc.tensor.matmul(ps[:], lhsT=maskrow[:1, sc * P:(sc + 1) * P],
                                 rhs=onesr[:1, :], start=False, stop=True)
                nc.scalar.activation(exp_c[:, sc, :], ps[:], AF.Exp)

            # compact copy delta + attention row sums (rr = 1/rowsum(delta))
            for tt in range(TT):
                ps = fp5.tile([P, 512], F32, tag="ps512", space="PSUM")
                for k in range(4):
                    nc.tensor.matmul(ps[:], lhsT=exp_c[:, k, tt * P:(tt + 1) * P],
                                     rhs=mcol[:, k, :], start=(k == 0),
                                     stop=(k == 3))
                nc.vector.reduce_sum(rr[:, tt:tt + 1], ps[:],
                                     axis=mybir.AxisListType.X)
                nc.scalar.copy(delta_tok[:, tt, :], ps[:])
            nc.vector.reciprocal(rr[:], rr[:])

            # attention value mix + output projection (feature-major)
            attnT = fa.tile([P, 4, T], BF16, tag="attnT")
            for dc in range(4):
                ps = fp.tile([P, T], F32, tag="ps256", space="PSUM")
                for sc in range(4):
                    nc.tensor.matmul(ps[:], lhsT=v_sb[:, sc, dc * P:(dc + 1) * P],
                                     rhs=exp_c[:, sc, :], start=(sc == 0),
                                     stop=(sc == 3))
                if dc % 2 == 0:
                    nc.vector.tensor_copy(attnT[:, dc, :], ps[:])
                else:
                    nc.scalar.copy(attnT[:, dc, :], ps[:])
            attn_oT = fa.tile([P, 4, T], BF16, tag="attn_oT")
            for ho in range(4):
                ps = fp.tile([P, T], F32, tag="ps256", space="PSUM")
                for k in range(4):
                    nc.tensor.matmul(ps[:], lhsT=woT[:, k, ho * P:(ho + 1) * P],
                                     rhs=attnT[:, k, :], start=(k == 0),
                                     stop=(k == 3))
                if ho % 2 == 0:
                    nc.vector.tensor_copy(attn_oT[:, ho, :], ps[:])
                else:
                    nc.scalar.copy(attn_oT[:, ho, :], ps[:])

            for tt in range(TT):
                for ho in range(4):
                    pst = fpb.tile([P, T], BF16, tag="psb", space="PSUM")
                    nc.tensor.transpose(pst[:, :P],
                                        attn_oT[:, ho, tt * P:(tt + 1) * P],
                                        ident_b[:])
                    if ho % 2 == 0:
                        nc.vector.tensor_copy(
                            attn_tok[:, tt, ho * P:(ho + 1) * P], pst[:, :P])
                    else:
                        nc.scalar.copy(attn_tok[:, tt, ho * P:(ho + 1) * P],
                                       pst[:, :P])
            for tt in range(TT):
                nc.vector.scalar_tensor_tensor(
                    out=attn_tok[:, tt, :], in0=attn_tok[:, tt, :],
                    scalar=rr[:, tt:tt + 1], in1=bo_tok[:],
                    op0=OP.mult, op1=OP.add)

        # ---------------- stage B: gates + FFN ----------------
        with ExitStack() as bctx:
            fb = bctx.enter_context(tc.tile_pool(name="fb", bufs=1))
            fbd = bctx.enter_context(tc.tile_pool(name="fbd", bufs=2))
            fp = bctx.enter_context(tc.tile_pool(name="fp2", bufs=4, space="PSUM"))
            fpb = bctx.enter_context(tc.tile_pool(name="fpb", bufs=4, space="PSUM"))

            # stage-1 LN stats for attn (gate path) and outs+attn (FFN path);
            # one Ln + one Exp for all four rstds.  ln1_g/ln1_b are folded
            # into wdn2 / nbddiff (gate) and W1 / b1 (FFN) on the host.
            negmu4 = fb.tile([P, 4], F32, tag="negmu4")
            ss4 = fb.tile([P, 4], F32, tag="ss4")
            rstd4 = fb.tile([P, 4], F32, tag="rstd4")
            res = fb.tile([P, TT, D], BF16, tag="res")
            for tt in range(TT):
                nc.vector.tensor_add(res[:, tt, :], outs_tok[:, tt, :],
                                     attn_tok[:, tt, :])
            for tt in range(TT):
                ln_stats(fb, fbd, attn_tok[:, tt, :], negmu4[:, tt:tt + 1],
                         ss4[:, tt:tt + 1], f"an{tt}")
                ln_stats(fb, fbd, res[:, tt, :], negmu4[:, 2 + tt:3 + tt],
                         ss4[:, 2 + tt:3 + tt], f"x{tt}")
            ln_rstd(rstd4[:], ss4[:])

            # gate logit ld = outs.wd1 + LN(attn).(g1*wd2) + const(folded)
            ld = fb.tile([P, TT], F32, tag="ld")
            x_tok = fb.tile([P, TT, D], BF16, tag="x_tok")
            for tt in range(TT):
                ta = fbd.tile([P, D], F32, tag="fscr", name="ta")
                nc.vector.tensor_scalar(out=ta[:], in0=attn_tok[:, tt, :],
                                        scalar1=negmu4[:, tt:tt + 1],
                                        scalar2=rstd4[:, tt:tt + 1],
                                        op0=OP.add, op1=OP.mult)
                lda = fb.tile([P, 1], F32, tag="lda")
                ldb = fb.tile([P, 1], F32, tag="ldb")
                scr = fbd.tile([P, D], F32, tag="fscr", name="ld_scr")
                nc.vector.scalar_tensor_tensor(out=scr[:], in0=outs_tok[:, tt, :],
                                               scalar=1.0, in1=wd1_tok[:],
                                               op0=OP.mult, op1=OP.mult,
                                               accum_out=lda[:])
                scr2 = fbd.tile([P, D], F32, tag="fscr", name="ld_scr2")
                nc.vector.scalar_tensor_tensor(out=scr2[:], in0=ta[:],
                                               scalar=1.0, in1=wdn2_tok[:],
                                               op0=OP.mult, op1=OP.mult,
                                               accum_out=ldb[:])
                nc.vector.tensor_add(ld[:, tt:tt + 1], lda[:], ldb[:])
                # x for FFN: (res - mu) * rstd  (g1/b1 folded into W1/b1)
                nc.vector.tensor_scalar(out=x_tok[:, tt, :], in0=res[:, tt, :],
                                        scalar1=negmu4[:, 2 + tt:3 + tt],
                                        scalar2=rstd4[:, 2 + tt:3 + tt],
                                        op0=OP.add, op1=OP.mult)
            # gates: cg = sigmoid(ld + bddiff) = 1/(1+exp(-ld-bddiff))
            et = fb.tile([P, TT], F32, tag="et")
            nc.scalar.activation(et[:], ld[:], AF.Exp, bias=nbddiff[:, :1],
                                 scale=-1.0)
            cg = fb.tile([P, TT], F32, tag="cg")
            nc.vector.tensor_scalar(out=cg[:], in0=et[:], scalar1=1.0,
                                    scalar2=None, op0=OP.add)
            nc.vector.reciprocal(cg[:], cg[:])
            nc.vector.tensor_scalar(out=gg[:], in0=cg[:], scalar1=-1.0,
                                    scalar2=1.0, op0=OP.mult, op1=OP.add)
            nc.vector.tensor_mul(cgrr[:], cg[:], rr[:])

            # FFN
            xT = fb.tile([P, 4, T], BF16, tag="xT")
            for tt in range(TT):
                for k in range(4):
                    pst = fpb.tile([P, T], BF16, tag="psb", space="PSUM")
                    nc.tensor.transpose(pst[:, :P],
                                        x_tok[:, tt, k * P:(k + 1) * P],
                                        ident_b[:])
                    if k % 2 == 0:
                        nc.vector.tensor_copy(xT[:, k, tt * P:(tt + 1) * P],
                                              pst[:, :P])
                    else:
                        nc.scalar.copy(xT[:, k, tt * P:(tt + 1) * P],
                                       pst[:, :P])
            h1T = fb.tile([P, 16, T], BF16, tag="h1T")
            for fc in range(16):
                ps = fp.tile([P, T], F32, tag="ps256", space="PSUM")
                for k in range(4):
                    nc.tensor.matmul(ps[:], lhsT=w1T[:, k, fc * P:(fc + 1) * P],
                                     rhs=xT[:, k, :], start=(k == 0),
                                     stop=(k == 3))
                nc.vector.tensor_scalar(out=h1T[:, fc, :], in0=ps[:],
                                        scalar1=b1_c[:, fc:fc + 1], scalar2=0.0,
                                        op0=OP.add, op1=OP.max)
            hT = fb.tile([P, 4, T], BF16, tag="hT")
            for ho in range(4):
                ps = fp.tile([P, T], F32, tag="ps256", space="PSUM")
                for fc in range(16):
                    nc.tensor.matmul(ps[:], lhsT=w2T[:, fc, ho * P:(ho + 1) * P],
                                     rhs=h1T[:, fc, :], start=(fc == 0),
                                     stop=(fc == 15))
                nc.vector.tensor_scalar(out=hT[:, ho, :], in0=ps[:],
                                        scalar1=b2_c[:, ho:ho + 1], scalar2=None,
                                        op0=OP.add)
            h_tok = fb.tile([P, TT, D], BF16, tag="h_tok")
            for tt in range(TT):
                for ho in range(4):
                    pst = fpb.tile([P, T], BF16, tag="psb", space="PSUM")
                    nc.tensor.transpose(pst[:, :P],
                                        hT[:, ho, tt * P:(tt + 1) * P],
                                        ident_b[:])
                    if ho % 2 == 0:
                        nc.vector.tensor_copy(
                            h_tok[:, tt, ho * P:(ho + 1) * P], pst[:, :P])
                    else:
                        nc.scalar.copy(h_tok[:, tt, ho * P:(ho + 1) * P],
                                       pst[:, :P])
            # LN2: stats, then in-place (h - mu) * rstd2 before fp8
            # quantization (ln2_g is folded into the embedding on the host)
            negmu2 = fb.tile([P, TT], F32, tag="negmu2")
            ss2 = fb.tile([P, TT], F32, tag="ss2")
            for tt in range(TT):
                ln_stats(fb, fbd, h_tok[:, tt, :], negmu2[:, tt:tt + 1],
                         ss2[:, tt:tt + 1], f"x2{tt}")
            ln_rstd(rstd2[:], ss2[:])
            for tt in range(TT):
                nc.vector.tensor_scalar(out=h_tok[:, tt, :],
                                        in0=h_tok[:, tt, :],
                                        scalar1=negmu2[:, tt:tt + 1],
                                        scalar2=rstd2[:, tt:tt + 1],
                                        op0=OP.add, op1=OP.mult)
            for tt in range(TT):
                for k in range(4):
                    pst = fpb.tile([P, T], BF16, tag="psb", space="PSUM")
                    nc.tensor.transpose(pst[:, :P],
                                        h_tok[:, tt, k * P:(k + 1) * P],
                                        ident_b[:])
                    nc.scalar.copy(x2T8[:, k, tt * P:(tt + 1) * P], pst[:, :P])

        fctx.close()

        # ================= vocab passes =================
        scrp = octx.enter_context(tc.tile_pool(name="scrp", bufs=2))
        outp = octx.enter_context(tc.tile_pool(name="outp", bufs=4))
        post = octx.enter_context(tc.tile_pool(name="post", bufs=1))
        mp = octx.enter_context(tc.tile_pool(name="mp", bufs=2, space="PSUM"))

        def vocab_mms(ps, c, tt):
            for k_sub, wk in enumerate(_subwidths(c)):
                pslice = ps[:, k_sub * 512:k_sub * 512 + wk]
                for i in range(2):
                    nc.tensor.matmul(
                        pslice,
                        lhsT=x2T8[:, 2 * i:2 * i + 2, tt * P:(tt + 1) * P],
                        rhs=wembs[c][:, 2 * i:2 * i + 2,
                                     k_sub * 512:k_sub * 512 + wk],
                        start=(i == 0), stop=(i == 1) and not has_b2g,
                        perf_mode=PM.DoubleRow)
                if has_b2g:
                    # + b2g @ W^T row (rarely needed; zero in practice)
                    nc.tensor.matmul(
                        pslice, lhsT=onesr[:1, tt * P:tt * P + P],
                        rhs=bw_row[:1, c * CHUNK + k_sub * 512:
                                   c * CHUNK + k_sub * 512 + wk],
                        start=False, stop=True)

        # ---- pass A: logits' -> exp(rstd2 * psum) -> row sums ----
        for c in range(NCHUNK):
            cw = _cw(c)
            for tt in range(TT):
                ps = mp.tile([P, CHUNK], F32, tag="bigps", space="PSUM")
                vocab_mms(ps, c, tt)
                scr = scrp.tile([P, CHUNK], BF16, tag="escr", name="escr")
                nc.scalar.activation(
                    scr[:, :cw], ps[:, :cw], AF.Exp,
                    accum_out=sv_parts[:, tt * NCHUNK + c:tt * NCHUNK + c + 1])

        # ---- ggsv = gg/sv;  compact fix exp side (all still exp-table) ----
        for tt in range(TT):
            nc.vector.reduce_sum(svs[:, tt:tt + 1],
                                 sv_parts[:, tt * NCHUNK:(tt + 1) * NCHUNK],
                                 axis=mybir.AxisListType.X)
        rsv = cpool.tile([P, TT], F32, tag="rsv")
        nc.vector.reciprocal(rsv[:], svs[:])
        nc.vector.tensor_mul(ggsv[:], gg[:], rsv[:])

        ev2 = post.tile([P, TT, NCOL], F32, tag="ev2")
        for tt in range(TT):
            ps = mp.tile([P, CHUNK], F32, tag="bigps", space="PSUM")
            for i in range(2):
                nc.tensor.matmul(
                    ps[:, :NCOL],
                    lhsT=x2T8[:, 2 * i:2 * i + 2, tt * P:(tt + 1) * P],
                    rhs=wcolsT[:, 2 * i:2 * i + 2, :],
                    start=(i == 0), stop=(i == 1) and not has_b2g,
                    perf_mode=PM.DoubleRow)
            if has_b2g:
                nc.tensor.matmul(ps[:, :NCOL], lhsT=onesr[:1, tt * P:tt * P + P],
                                 rhs=bwc_row[:1, :], start=False, stop=True)
            nc.scalar.activation(ev2[:, tt, :], ps[:, :NCOL], AF.Exp)
        # val = ggsv*e + cgrr*delta  (DVE only)
        val2 = post.tile([P, TT, NCOL], BF16, tag="val2")
        for tt in range(TT):
            nc.vector.tensor_scalar(out=ev2[:, tt, :], in0=ev2[:, tt, :],
                                    scalar1=ggsv[:, tt:tt + 1], scalar2=None,
                                    op0=OP.mult)
            nc.vector.scalar_tensor_tensor(out=val2[:, tt, :],
                                           in0=delta_tok[:, tt, :],
                                           scalar=cgrr[:, tt:tt + 1],
                                           in1=ev2[:, tt, :],
                                           op0=OP.mult, op1=OP.add)

        # ---- Ln cluster: rowconst + fix output ----
        nc.scalar.activation(rowconst[:], ggsv[:], AF.Ln)
        fix_sb = post.tile([P, TT, NCOL], BF16, tag="fix_sb")
        for tt in range(TT):
            nc.scalar.activation(fix_sb[:, tt, :], val2[:, tt, :], AF.Ln,
                                 bias=eps_log_c[:, :1])
            nc.sync.dma_start(fix_r[:, tt, :], fix_sb[:, tt, :])

        # ---- pass B: recompute logits', fused scale+add, store bf16 ----
        # alternate the adds over DVE / ACT
        eng_cycle = ["dve", "act"]
        for c in range(NCHUNK):
            cw = _cw(c)
            for tt in range(TT):
                ps = mp.tile([P, CHUNK], F32, tag="bigps", space="PSUM")
                vocab_mms(ps, c, tt)
                ot = outp.tile([P, CHUNK], BF16, tag="out_sb")
                eng = eng_cycle[(c * TT + tt) % len(eng_cycle)]
                if eng == "dve":
                    nc.vector.tensor_scalar(out=ot[:, :cw], in0=ps[:, :cw],
                                            scalar1=rowconst[:, tt:tt + 1],
                                            scalar2=None, op0=OP.add)
                elif eng == "pool":
                    nc.gpsimd.tensor_scalar(out=ot[:, :cw], in0=ps[:, :cw],
                                            scalar1=rowconst[:, tt:tt + 1],
                                            scalar2=None, op0=OP.add)
                else:
                    nc.scalar.activation(ot[:, :cw], ps[:, :cw], AF.Identity,
                                         bias=rowconst[:, tt:tt + 1])
                nc.sync.dma_start(out_r[:, tt, c * CHUNK:c * CHUNK + cw],
                                  ot[:, :cw])

    nc.compile()
    return nc


def _tile_wemb8(w_emb):
    wp = np.zeros((NCHUNK * CHUNK, D), F8)
    wp[:V] = w_emb.astype(F8)
    wt = wp.reshape(NCHUNK, CHUNK, 4, P).transpose(0, 3, 2, 1)
    return np.ascontiguousarray(wt.reshape(NCHUNK, P, 4 * CHUNK))


def _prep(inputs):
    g = {k: np.asarray(v) for k, v in inputs.items()}
    f32 = np.float32

    # folds:
    #   ln2_g into the embedding; ln2_b via optional b2g@W^T row
    #   ln1_g/ln1_b into W1/b1 (FFN) and into wd2/bd (gate path)
    wg = g["W_emb"].astype(f32) * g["ln2_g"].astype(f32)[None, :]
    b2g = g["ln2_b"].astype(f32)
    has_b2g = bool(np.any(b2g != 0.0))
    g1 = g["ln1_g"].astype(f32)
    b1g = g["ln1_b"].astype(f32)
    w1f = g["W1"].astype(f32) * g1[None, :]          # [F, D] * g1[d]
    b1f = g["b1"].astype(f32) + g["W1"].astype(f32) @ b1g
    wd = g["Wd"].astype(f32)
    wd1 = wd[1, :D] - wd[0, :D]
    wd2 = wd[1, D:] - wd[0, D:]
    gate_const = float((wd2 * b1g).sum() + g["bd"][1] - g["bd"][0])

    shared = {
        "wqT": np.ascontiguousarray(g["Wq"].T.astype(BF)),
        "wkT": np.ascontiguousarray(g["Wk"].T.astype(BF)),
        "wvT": np.ascontiguousarray(g["Wv"].T.astype(BF)),
        "woT": np.ascontiguousarray(g["Wo"].T.astype(BF)),
        "w1T": np.ascontiguousarray(w1f.T.astype(F8)),
        "w2T": np.ascontiguousarray(g["W2"].T.astype(F8)),
        "wemb8": _tile_wemb8(wg),
        "bq_c": np.ascontiguousarray(g["bq"].astype(f32).reshape(4, P).T),
        "bk_c": np.ascontiguousarray(g["bk"].astype(f32).reshape(4, P).T),
        "bv_row": g["bv"].astype(BF)[None, :],
        "bo_tok": np.tile(g["bo"].astype(BF), (P, 1)),
        "b1_c": np.ascontiguousarray(b1f.reshape(16, P).T),
        "b2_c": np.ascontiguousarray(g["b2"].astype(f32).reshape(4, P).T),
        "wd1_tok": np.tile(wd1.astype(BF), (P, 1)),
        "wdn2_tok": np.tile((wd2 * g1).astype(BF), (P, 1)),
        "nbddiff": np.full((P, 1), -gate_const, f32),
        "ones_row": np.ones((1, T), BF),
    }
    if has_b2g:
        bw = np.zeros((1, NCHUNK * CHUNK), f32)
        bw[0, :V] = b2g @ g["W_emb"].astype(f32).T

    cs = g["copy_seq"].astype(np.int64)          # [S, B]
    mm_ = g["mem_mask"].astype(bool)             # [B, S]
    outs = g["outs"].astype(f32)                 # [T, B, D]
    mem = g["mem"].astype(f32)                   # [S, B, D]

    per_core = []
    cols_list = []
    for b in range(B):
        idx = cs[:, b]
        cols, inv = np.unique(idx, return_inverse=True)
        ncols = len(cols)
        assert ncols <= NCOL
        mcol = np.zeros((S, NCOL), f32)
        mcol[np.arange(S), inv] = 1.0
        wcols = np.zeros((NCOL, D), f32)
        wcols[:ncols] = wg[cols]
        maskrow = np.where(mm_[b], NEG, 0.0).astype(f32)
        pc = {
            "outsT": np.ascontiguousarray(outs[:, b, :].T.astype(BF)),
            "outs_tok": np.ascontiguousarray(outs[:, b, :].astype(BF)),
            "memT": np.ascontiguousarray(mem[:, b, :].T.astype(BF)),
            "maskrow": maskrow[None, :].astype(BF),
            "mcol": mcol.astype(F8),
            "wcolsT": np.ascontiguousarray(wcols.T.astype(F8)),
        }
        if has_b2g:
            pc["bw_row"] = bw
            bwc = np.zeros((1, NCOL), f32)
            bwc[0, :ncols] = b2g @ g["W_emb"].astype(f32)[cols].T
            pc["bwcol_row"] = bwc
        per_core.append(pc)
        cols_list.append(cols)
    return shared, per_core, cols_list, has_b2g


def kernel(**inputs):
    shared, per_core, cols_list, has_b2g = _prep(inputs)
    key = ("nc", has_b2g)
    if key not in _CACHE:
        _CACHE[key] = _build(has_b2g)
    nc = _CACHE[key]
    in_maps = [{**shared, **pc} for pc in per_core]
    res = run_bass_kernel_spmd(nc, in_maps, core_ids=list(range(B)))
    outs_l = []
    for b, r in enumerate(res.results):
        out = r["out"].astype(np.float32)        # [T, V]
        fix = r["fix"].astype(np.float32)        # [T, NCOL]
        cols = cols_list[b]
        out[:, cols] = fix[:, :len(cols)]
        outs_l.append(out)
    return np.stack(outs_l, axis=1)

